# revision 4
# baseline (speedup 1.0000x reference)
"""Butterfly permuter kernel for Trainium2 (8 NeuronCores, SPMD data-parallel).

The reference applies 10 butterfly rotation stages along the feature axis
(dim=1024) of x [16384, 1024].  Each row is transformed independently, and the
10 stages compose into a single dense 1024x1024 orthogonal matrix R with
y_rows = x_rows @ R.  We compute R on the host in float64 from `angles`, then
run a tiled matmul on each core:

  per core: x_shard [2048, 1024]
  - DMA x in 2 MiB megatiles [128 part, 4096] (4 row-subtiles of 128 tokens)
  - PE-transpose each [128 tok, 128 dim] block (float32r, via identity) to get
    X^T blocks (contraction dim on partitions), evacuate PSUM->SBUF on ScalarE
  - 16 accumulating float32r matmuls per subtile: psum_y[jh] += XT_kb^T @ R_kb
    (float32r streams 1 cycle/row at N=512 - full PE rate, ~fp32 storage)
  - evacuate y PSUM->SBUF on VectorE, DMA out 2 MiB megatiles

Inputs arrive full-size; sharding is across the token axis (2048 rows/core).
"""

import numpy as np

import concourse.bass as bass
import concourse.mybir as mybir
import concourse.tile as tile
from concourse import bacc
from concourse.bass_utils import run_bass_kernel_spmd

N_CORES = 8
DIM = 1024
NUM_STAGES = 10
N_TOKENS = 16384
TOK_PER_CORE = N_TOKENS // N_CORES  # 2048
SUB = 128  # tokens per subtile (partition dim)
SUBTILES_PER_MEGA = 4
MEGA_ROWS = SUB * SUBTILES_PER_MEGA  # 512 tokens per DMA megatile
N_MEGA = TOK_PER_CORE // MEGA_ROWS  # 4
KB = DIM // 128  # 8 contraction blocks

F32 = mybir.dt.float32
F32R = mybir.dt.float32r


def compose_transform(angles: np.ndarray) -> np.ndarray:
    """Compose the 10 butterfly stages into R (float32) with y = x @ R."""
    y = np.eye(DIM, dtype=np.float64)
    a = np.asarray(angles, dtype=np.float64)
    for s in range(NUM_STAGES):
        span = 2 ** (s + 1)
        half = span // 2
        y = y.reshape(-1, DIM // span, span)
        left, right = y[..., :half], y[..., half:]
        th = a[s].reshape(1, DIM // span, half)
        c, sn = np.cos(th), np.sin(th)
        y = np.concatenate([c * left + sn * right, -sn * left + c * right], -1)
        y = y.reshape(-1, DIM)
    # row t of y is transform(e_t), so transform(x) = x @ y
    return np.ascontiguousarray(y, dtype=np.float32)


def build_bass():
    nc = bacc.Bacc(None, target_bir_lowering=False)
    x = nc.dram_tensor("x", [TOK_PER_CORE, DIM], F32, kind="ExternalInput")
    w = nc.dram_tensor("w", [DIM, DIM], F32, kind="ExternalInput")
    ident = nc.dram_tensor("ident", [128, 128], F32, kind="ExternalInput")
    y = nc.dram_tensor("y", [TOK_PER_CORE, DIM], F32, kind="ExternalOutput")

    with tile.TileContext(nc) as tc:
        with (
            tc.tile_pool(name="const", bufs=1) as const_pool,
            tc.tile_pool(name="xin", bufs=2) as xin_pool,
            tc.tile_pool(name="xt", bufs=3) as xt_pool,
            tc.tile_pool(name="yout", bufs=2) as yout_pool,
            tc.tile_pool(name="pst", bufs=4, space="PSUM") as pst_pool,
            tc.tile_pool(name="psy", bufs=4, space="PSUM") as psy_pool,
        ):
            # Preamble: load W (8 k-blocks side by side), rounding to f32r via
            # a DVE copy (walrus requires f32r matmul inputs to be produced by
            # a rounding instruction), and the identity.
            w_sbr = const_pool.tile([128, KB * DIM], F32R, name="w_sbr")
            for kb in range(KB):
                w_stage = xin_pool.tile([128, DIM], F32, name="w_stage",
                                        tag="w_stage")
                nc.sync.dma_start(w_stage[:], w[kb * 128 : (kb + 1) * 128, :])
                nc.vector.tensor_copy(w_sbr[:, kb * DIM : (kb + 1) * DIM],
                                      w_stage[:])
            ident_sb = const_pool.tile([128, 128], F32, name="ident_sb")
            nc.sync.dma_start(ident_sb[:], ident[:])

            for m in range(N_MEGA):
                r0 = m * MEGA_ROWS
                x_mega = xin_pool.tile([128, SUBTILES_PER_MEGA * DIM], F32,
                                       name="x_mega")
                nc.sync.dma_start(
                    x_mega.rearrange("p (s c) -> p s c", c=DIM),
                    x[r0 : r0 + MEGA_ROWS, :].rearrange("(s p) c -> p s c", p=128),
                )
                y_mega = yout_pool.tile([128, SUBTILES_PER_MEGA * DIM], F32,
                                        name="y_mega")
                for s in range(SUBTILES_PER_MEGA):
                    xcol = s * DIM
                    # --- transpose 8 [128,128] blocks via PE ---
                    ps_t0 = pst_pool.tile([128, 512], F32, name="ps_t0", tag="ps_t")
                    ps_t1 = pst_pool.tile([128, 512], F32, name="ps_t1", tag="ps_t")
                    for kb in range(KB):
                        dst = ps_t0 if kb < 4 else ps_t1
                        j = (kb % 4) * 128
                        nc.tensor.transpose(
                            dst[:, j : j + 128],
                            x_mega[:, xcol + kb * 128 : xcol + (kb + 1) * 128],
                            ident_sb,
                        )
                    xt = xt_pool.tile([128, DIM], F32R, name="xt")
                    nc.scalar.copy(xt[:, :512], ps_t0[:])
                    nc.scalar.copy(xt[:, 512:], ps_t1[:])
                    # --- 16 accumulating matmuls: y = X @ R ---
                    ps_y0 = psy_pool.tile([128, 512], F32, name="ps_y0", tag="ps_y")
                    ps_y1 = psy_pool.tile([128, 512], F32, name="ps_y1", tag="ps_y")
                    for jh, ps_y in ((0, ps_y0), (1, ps_y1)):
                        for kb in range(KB):
                            nc.tensor.matmul(
                                ps_y[:],
                                xt[:, kb * 128 : (kb + 1) * 128],
                                w_sbr[:, kb * DIM + jh * 512 : kb * DIM + jh * 512 + 512],
                                start=(kb == 0),
                                stop=(kb == KB - 1),
                            )
                    nc.vector.tensor_copy(y_mega[:, xcol : xcol + 512], ps_y0[:])
                    nc.vector.tensor_copy(y_mega[:, xcol + 512 : xcol + DIM], ps_y1[:])
                nc.sync.dma_start(
                    y[r0 : r0 + MEGA_ROWS, :].rearrange("(s p) c -> p s c", p=128),
                    y_mega.rearrange("p (s c) -> p s c", c=DIM),
                )
    nc.compile()
    return nc


_NC_CACHE = None


def _get_nc():
    global _NC_CACHE
    if _NC_CACHE is None:
        _NC_CACHE = build_bass()
    return _NC_CACHE


def run(x: np.ndarray, angles: np.ndarray, trace: bool = False):
    """Run on 8 cores; returns (y_full, BassKernelResults)."""
    x = np.ascontiguousarray(np.asarray(x, dtype=np.float32))
    w = compose_transform(angles)
    ident = np.eye(128, dtype=np.float32)
    nc = _get_nc()
    in_maps = []
    for c in range(N_CORES):
        in_maps.append(
            {
                "x": x[c * TOK_PER_CORE : (c + 1) * TOK_PER_CORE],
                "w": w,
                "ident": ident,
            }
        )
    res = run_bass_kernel_spmd(
        nc, in_maps, core_ids=list(range(N_CORES)), trace=trace
    )
    y = np.concatenate([res.results[c]["y"] for c in range(N_CORES)], axis=0)
    return y, res


def kernel(x: np.ndarray, angles: np.ndarray) -> np.ndarray:
    y, _ = run(x, angles, trace=False)
    return y


# revision 16
# speedup vs baseline: 91.8534x; 91.8534x over previous
"""Butterfly permuter kernel for Trainium2 (8 NeuronCores, SPMD data-parallel).

The reference applies 10 butterfly rotation stages along the feature axis
(dim=1024) of x [16384, 1024].  Each row is transformed independently, and the
10 stages compose into a single dense 1024x1024 orthogonal matrix R with
y_rows = x_rows @ R.  We compute R on the host in float64 from `angles`, then
run a tiled matmul on each core:

  per core: x_shard [2048, 1024]
  - DMA x in 2 MiB megatiles [128 part, 4096] (4 row-subtiles of 128 tokens)
  - PE-transpose each [128 tok, 128 dim] block (float32r, via identity) to get
    X^T blocks (contraction dim on partitions), evacuate PSUM->SBUF on ScalarE
  - 16 accumulating float32r matmuls per subtile: psum_y[jh] += XT_kb^T @ R_kb
    (float32r streams 1 cycle/row at N=512 - full PE rate, ~fp32 storage)
  - evacuate y PSUM->SBUF on VectorE, DMA out 2 MiB megatiles

Inputs arrive full-size; sharding is across the token axis (2048 rows/core).
"""

import numpy as np

import concourse.bass as bass
import concourse.mybir as mybir
import concourse.tile as tile
from concourse import bacc
from concourse.bass_utils import run_bass_kernel_spmd

N_CORES = 8
DIM = 1024
NUM_STAGES = 10
N_TOKENS = 16384
TOK_PER_CORE = N_TOKENS // N_CORES  # 2048
SUB = 128  # tokens per subtile (partition dim)
SUBTILES_PER_MEGA = 4
MEGA_ROWS = SUB * SUBTILES_PER_MEGA  # 512 tokens per DMA megatile
N_MEGA = TOK_PER_CORE // MEGA_ROWS  # 4
KB = DIM // 128  # 8 contraction blocks

F32 = mybir.dt.float32
F32R = mybir.dt.float32r


def compose_transform(angles: np.ndarray) -> np.ndarray:
    """Compose the 10 butterfly stages into R (float32) with y = x @ R."""
    y = np.eye(DIM, dtype=np.float64)
    a = np.asarray(angles, dtype=np.float64)
    for s in range(NUM_STAGES):
        span = 2 ** (s + 1)
        half = span // 2
        y = y.reshape(-1, DIM // span, span)
        left, right = y[..., :half], y[..., half:]
        th = a[s].reshape(1, DIM // span, half)
        c, sn = np.cos(th), np.sin(th)
        y = np.concatenate([c * left + sn * right, -sn * left + c * right], -1)
        y = y.reshape(-1, DIM)
    # row t of y is transform(e_t), so transform(x) = x @ y
    return np.ascontiguousarray(y, dtype=np.float32)


def build_bass(reps: int = 1):
    """reps>1 repeats the whole pipeline in one NEFF (for marginal timing)."""
    nc = bacc.Bacc(None, target_bir_lowering=False)
    x = nc.dram_tensor("x", [TOK_PER_CORE, DIM], F32, kind="ExternalInput")
    w = nc.dram_tensor("w", [DIM, DIM], F32, kind="ExternalInput")
    ident = nc.dram_tensor("ident", [128, 128], F32, kind="ExternalInput")
    y = nc.dram_tensor("y", [TOK_PER_CORE, DIM], F32, kind="ExternalOutput")

    n_sub = N_MEGA * SUBTILES_PER_MEGA  # 16 subtiles of 128 tokens

    # Variable-size DMA chunking (in units of 128-token subtiles): small
    # chunks at the start for a fast pipeline ramp, small at the end for a
    # short drain; 2-subtile (1 MiB) chunks in steady state.
    in_chunks = [1, 1, 2, 2, 2, 2, 2, 2, 2]
    out_chunks = [2, 2, 2, 2, 2, 2, 2, 1, 1]
    assert sum(in_chunks) == n_sub and sum(out_chunks) == n_sub
    in_start = [sum(in_chunks[:i]) for i in range(len(in_chunks))]
    out_start = [sum(out_chunks[:i]) for i in range(len(out_chunks))]
    sub_to_in_chunk = {}
    for ci, (st, ln) in enumerate(zip(in_start, in_chunks)):
        for s in range(st, st + ln):
            sub_to_in_chunk[s] = ci
    sub_to_out_chunk = {}
    for ci, (st, ln) in enumerate(zip(out_start, out_chunks)):
        for s in range(st, st + ln):
            sub_to_out_chunk[s] = ci

    with tile.TileContext(nc) as tc:
        with (
            tc.tile_pool(name="const", bufs=1) as const_pool,
            tc.tile_pool(name="wstage", bufs=3) as wstage_pool,
            tc.tile_pool(name="xin", bufs=3) as xin_pool,
            tc.tile_pool(name="xt", bufs=3) as xt_pool,
            tc.tile_pool(name="yout", bufs=3) as yout_pool,
            tc.tile_pool(name="pst", bufs=4, space="PSUM") as pst_pool,
            tc.tile_pool(name="psy", bufs=4, space="PSUM") as psy_pool,
        ):
            # identity goes via the SWDGE ring; the SP ring starts with the
            # first x chunk; W streams on the ACT ring in parallel.
            ident_sb = const_pool.tile([128, 128], F32, name="ident_sb")
            nc.gpsimd.dma_start(ident_sb[:], ident[:])

            x_tiles = [None] * len(in_chunks)  # chunk idx -> (tile, start_sub)
            y_tiles = [None] * len(out_chunks)

            def load_chunk(ci):
                st, ln = in_start[ci], in_chunks[ci]
                x_tile = xin_pool.tile([128, ln * DIM], F32, name="x_chunk",
                                       tag="x_chunk",
                                       padded_shape=[128, 2 * DIM])
                r0 = st * SUB
                nc.sync.dma_start(
                    x_tile[:, : ln * DIM].rearrange("p (s c) -> p s c", c=DIM),
                    x[r0 : r0 + ln * SUB, :].rearrange("(s p) c -> p s c", p=128),
                )
                x_tiles[ci] = x_tile

            load_chunk(0)
            first_load_done = True

            # W: DMA [jh][kb] blocks of [128,512] (j-half-major so the first
            # 2 MiB unblocks the first matmul group) on the ACT HWDGE ring,
            # then round fp32 -> f32r on DVE (walrus requires f32r matmul
            # inputs to come from a rounding instruction).
            w_sbr = const_pool.tile([128, KB * DIM], F32R, name="w_sbr")

            def w_off(jh, kb):
                return (jh * KB + kb) * 512

            for jh in range(2):
                for kb in range(KB):
                    w_stage = wstage_pool.tile([128, 512], F32, name="w_stage",
                                               tag="w_stage")
                    nc.sync.dma_start(
                        w_stage[:],
                        w[kb * 128 : (kb + 1) * 128, jh * 512 : (jh + 1) * 512],
                    )
                    off = w_off(jh, kb)
                    nc.vector.tensor_copy(w_sbr[:, off : off + 512], w_stage[:])

            xts = [None] * n_sub

            def emit_transpose(s):
                ci = sub_to_in_chunk[s]
                xcol = (s - in_start[ci]) * DIM
                x_tile = x_tiles[ci]
                ps_t0 = pst_pool.tile([128, 512], F32, name="ps_t0", tag="ps_t")
                ps_t1 = pst_pool.tile([128, 512], F32, name="ps_t1", tag="ps_t")
                for kb in range(KB):
                    dst = ps_t0 if kb < 4 else ps_t1
                    j = (kb % 4) * 128
                    nc.tensor.transpose(
                        dst[:, j : j + 128],
                        x_tile[:, xcol + kb * 128 : xcol + (kb + 1) * 128],
                        ident_sb,
                    )
                xt = xt_pool.tile([128, DIM], F32R, name="xt", tag="xt")
                nc.scalar.copy(xt[:, :512], ps_t0[:])
                nc.scalar.copy(xt[:, 512:], ps_t1[:])
                xts[s] = xt

            def emit_matmul(s):
                co = sub_to_out_chunk[s]
                st, ln = out_start[co], out_chunks[co]
                if s == st:
                    y_tiles[co] = yout_pool.tile(
                        [128, ln * DIM], F32, name="y_chunk", tag="y_chunk",
                        padded_shape=[128, 2 * DIM],
                    )
                y_tile = y_tiles[co]
                ycol = (s - st) * DIM
                xt = xts[s]
                ps_y0 = psy_pool.tile([128, 512], F32, name="ps_y0", tag="ps_y")
                ps_y1 = psy_pool.tile([128, 512], F32, name="ps_y1", tag="ps_y")
                for jh, ps_y in ((0, ps_y0), (1, ps_y1)):
                    for kb in range(KB):
                        off = (jh * KB + kb) * 512
                        nc.tensor.matmul(
                            ps_y[:],
                            xt[:, kb * 128 : (kb + 1) * 128],
                            w_sbr[:, off : off + 512],
                            start=(kb == 0),
                            stop=(kb == KB - 1),
                        )
                nc.vector.tensor_copy(y_tile[:, ycol : ycol + 512], ps_y0[:])
                nc.vector.tensor_copy(y_tile[:, ycol + 512 : ycol + DIM], ps_y1[:])
                if s == st + ln - 1:
                    r0 = st * SUB
                    # y stores go out on the ACT HWDGE ring so they don't
                    # queue ahead of later x loads on the SP ring.
                    nc.scalar.dma_start(
                        y[r0 : r0 + ln * SUB, :].rearrange("(s p) c -> p s c", p=128),
                        y_tile[:, : ln * DIM].rearrange("p (s c) -> p s c", c=DIM),
                    )

            # Skewed software pipeline: transposes run one subtile ahead of
            # the matmuls so the PE never waits on the ScalarE PSUM->SBUF
            # evacuation of its own transpose outputs.
            for _rep in range(reps):
                if not first_load_done:
                    load_chunk(0)
                first_load_done = False
                emit_transpose(0)
                for s in range(n_sub):
                    nxt = s + 1
                    if nxt < n_sub:
                        ci = sub_to_in_chunk[nxt]
                        if nxt == in_start[ci]:
                            load_chunk(ci)
                        emit_transpose(nxt)
                    emit_matmul(s)
    nc.compile()
    return nc


_NC_CACHE = None


def _get_nc():
    global _NC_CACHE
    if _NC_CACHE is None:
        _NC_CACHE = build_bass()
    return _NC_CACHE


def run(x: np.ndarray, angles: np.ndarray, trace: bool = False):
    """Run on 8 cores; returns (y_full, BassKernelResults)."""
    x = np.ascontiguousarray(np.asarray(x, dtype=np.float32))
    w = compose_transform(angles)
    ident = np.eye(128, dtype=np.float32)
    nc = _get_nc()
    in_maps = []
    for c in range(N_CORES):
        in_maps.append(
            {
                "x": x[c * TOK_PER_CORE : (c + 1) * TOK_PER_CORE],
                "w": w,
                "ident": ident,
            }
        )
    res = run_bass_kernel_spmd(
        nc, in_maps, core_ids=list(range(N_CORES)), trace=trace
    )
    y = np.concatenate([res.results[c]["y"] for c in range(N_CORES)], axis=0)
    return y, res


def kernel(x: np.ndarray, angles: np.ndarray) -> np.ndarray:
    y, _ = run(x, angles, trace=False)
    return y
